# revision 10
# baseline (speedup 1.0000x reference)
"""Distributed Bass attention kernel for trn2 (8 NeuronCores).

Problem: B=4,H=16,T=2048,D=128 attention w/ Q/K/V linear projections.
  qp = q@Wq.T+bq ; kp = k@Wk.T+bk ; vp = v@Wv.T+bv
  S = qp@kp.T/sqrt(128); S = where(mask==1, -1e-9, S); P=softmax(S); out = P@vp

Key identities:
  - masked logit -1e-9 ~= 0  =>  P_unnorm[i,j] = exp(S[i,j]*(1-m)) (exp(0)=1).
  - global shift C=ln(8): P' = exp(S*(1-m) - C) = P/8 cancels in softmax;
    masked entries become exactly 0.125 (representable in low precision).
  - out row i = (P' @ vpx)[i,:]/l[i], l via ones-column appended to vp.
  - bias bv folded into vp via a K=1 rank-1 matmul accumulation
    (ones[1,128].T @ bv4[1,512]); out = P_norm@(vp+bv) == P_norm@vp + bv.

Sharding: 64 (b,h) slabs -> 8 per core (head/data parallel, no collectives).

Per-core dataflow (host pre-transposes+casts q/k/v to [d,t] bf16):
  - qpT[e,t] = wqt.T @ qT (+bq per-partition during PSUM->SBUF activation)
  - vp[t,e] natural via lhsT=vT tile, rhs=wvt; +bv via rank-1 matmul;
    ones col appended -> vpx bf16 [128, 16*129]
  - S TRANSPOSED in [128,1024] 2-bank PSUM groups: ST[j,i] = kpT_j.T @ qpT_i
  - DVE: smt = ST * (1-m)^T (mask fp8 0/1, exact), bf16 out, [128,2048] pairs
  - Scalar: P^T = exp(smt*scale - C) -> bf16 pt, one [128,2048] instr per pair
  - AV: out[i,0:129] = sum_jt PT_tile.T @ vpx_tile (l in col 128)
  - epilogue: rl=1/l (DVE), out = O*rl on Scalar (PSUM->SBUF), chunked DMA out
"""

import numpy as np
import ml_dtypes

import sys
sys.path.insert(0, "/opt/trn_rl_repo")

from concourse import bacc, bass, mybir
from concourse.tile import TileContext
from concourse.bass_utils import run_bass_kernel_spmd

B, H, T, D = 4, 16, 2048, 128
NCORES = 8
SPC = (B * H) // NCORES  # 8 slabs per core
ROWS = SPC * T
NT = T // 128  # 16 j-tiles
IC = 1024  # i-chunk size
NCI = T // IC  # 2
SCALE = 1.0 / np.sqrt(D)
C_SHIFT = float(np.log(8.0))

F32 = mybir.dt.float32
BF16 = mybir.dt.bfloat16
FP8 = mybir.dt.float8e4
AF = mybir.ActivationFunctionType


def _build_nc():
    nc = bacc.Bacc(target_bir_lowering=False, trn_type="TRN2")

    qt_d = nc.declare_dram_parameter("qt", [SPC * 128, T], BF16, isOutput=False)
    kt_d = nc.declare_dram_parameter("kt", [SPC * 128, T], BF16, isOutput=False)
    vt_d = nc.declare_dram_parameter("vt", [SPC * 128, T], BF16, isOutput=False)
    mmt_d = nc.declare_dram_parameter("mmt", [T, T], FP8, isOutput=False)
    wqt_d = nc.declare_dram_parameter("wqt", [D, D], BF16, isOutput=False)
    wkt_d = nc.declare_dram_parameter("wkt", [D, D], BF16, isOutput=False)
    wvt_d = nc.declare_dram_parameter("wvt", [D, D], BF16, isOutput=False)
    bqc_d = nc.declare_dram_parameter("bqc", [D, 1], F32, isOutput=False)
    bkc_d = nc.declare_dram_parameter("bkc", [D, 1], F32, isOutput=False)
    bv4_d = nc.declare_dram_parameter("bv4", [1, 512], BF16, isOutput=False)
    # out blocks: row = (s*NCI + ic)*128 + p, col = t*128 + e  (host reshapes)
    out_d = nc.declare_dram_parameter(
        "out", [SPC * NCI * 128, IC // 128 * D], F32, isOutput=True
    )

    with TileContext(nc) as tc:
        with (
            tc.tile_pool(name="const", bufs=1) as const_pool,
            tc.tile_pool(name="mmt", bufs=1) as mmt_pool,
            tc.tile_pool(name="qkvt", bufs=2) as qkvt_pool,
            tc.tile_pool(name="proj", bufs=2) as proj_pool,
            tc.tile_pool(name="vpx", bufs=2) as vpx_pool,
            tc.tile_pool(name="sm", bufs=3) as sm_pool,
            tc.tile_pool(name="pt", bufs=2) as pt_pool,
            tc.tile_pool(name="fin", bufs=2) as fin_pool,
            tc.tile_pool(name="rl", bufs=8) as rl_pool,
            tc.tile_pool(name="pj_ps", bufs=1, space="PSUM") as pjps_pool,
            tc.tile_pool(name="s_ps", bufs=2, space="PSUM") as sps_pool,
            tc.tile_pool(name="o_ps", bufs=2, space="PSUM") as ops_pool,
        ):
            # ---- constants (once per core) ----
            wqt = const_pool.tile([128, 128], BF16, tag="wqt")
            nc.sync.dma_start(out=wqt[:, :], in_=wqt_d[:, :])
            wkt = const_pool.tile([128, 128], BF16, tag="wkt")
            nc.sync.dma_start(out=wkt[:, :], in_=wkt_d[:, :])
            wvt = const_pool.tile([128, 128], BF16, tag="wvt")
            nc.sync.dma_start(out=wvt[:, :], in_=wvt_d[:, :])
            bqc = const_pool.tile([128, 1], F32, tag="bqc")
            nc.sync.dma_start(out=bqc[:, :], in_=bqc_d[:, :])
            bkc = const_pool.tile([128, 1], F32, tag="bkc")
            nc.sync.dma_start(out=bkc[:, :], in_=bkc_d[:, :])
            bv4 = const_pool.tile([1, 512], BF16, tag="bv4")
            nc.sync.dma_start(out=bv4[:, :], in_=bv4_d[:, :])
            ones1 = const_pool.tile([1, 128], BF16, tag="ones1")
            nc.vector.memset(ones1[:, :], 1.0)
            negc = const_pool.tile([128, 1], F32, tag="negc")
            nc.vector.memset(negc[:, :], -C_SHIFT)

            # slab-0 q/k/v loads issued BEFORE the big mask load so the
            # proj->S->mul pipeline starts immediately
            qkv0 = []
            for name, src in (("qT", qt_d), ("kT", kt_d), ("vT", vt_d)):
                t0 = qkvt_pool.tile([128, T], BF16, tag=name)
                nc.sync.dma_start(out=t0[:, :], in_=src[0:128, :])
                qkv0.append(t0)

            # transposed mask multiplier (1-mask).T, fp8 (0/1 exact), [j, i]
            mmt = mmt_pool.tile([128, NT * T], FP8, tag="mmt")
            for jt in range(NT):
                nc.sync.dma_start(
                    out=mmt[:, jt * T : (jt + 1) * T],
                    in_=mmt_d[jt * 128 : (jt + 1) * 128, :],
                )

            # ---- software-pipelined slab phases ----
            def load(s):
                if s == 0:
                    return qkv0
                tiles = []
                for name, src in (("qT", qt_d), ("kT", kt_d), ("vT", vt_d)):
                    t = qkvt_pool.tile([128, T], BF16, tag=name)
                    nc.sync.dma_start(
                        out=t[:, :], in_=src[s * 128 : (s + 1) * 128, :]
                    )
                    tiles.append(t)
                return tiles

            def proj(qT, kT):
                qpT = proj_pool.tile([128, T], BF16, tag="qpT")
                kpT = proj_pool.tile([128, T], BF16, tag="kpT")
                for srcT, w, bias, dst in ((qT, wqt, bqc, qpT), (kT, wkt, bkc, kpT)):
                    for c in range(T // 1024):
                        pps = pjps_pool.tile([128, 1024], F32, tag="pj")
                        for h in range(2):
                            nc.tensor.matmul(
                                pps[:, h * 512 : (h + 1) * 512],
                                w[:, :],
                                srcT[:, c * 1024 + h * 512 : c * 1024 + (h + 1) * 512],
                                start=True,
                                stop=True,
                            )
                        nc.scalar.activation(
                            dst[:, c * 1024 : (c + 1) * 1024],
                            pps[:, :],
                            AF.Identity,
                            bias=bias[:, :],
                            scale=1.0,
                        )
                return qpT, kpT

            def vproj(vT):
                # vpx: 16 blocks [128(t), 129] bf16; col 128 = 1.0
                vpx = vpx_pool.tile([128, NT * 129], BF16, tag="vpx")
                nc.gpsimd.memset(vpx[:, :], 1.0)
                vpxv = vpx[:, :].rearrange("p (j n) -> p j n", j=NT)
                for b8 in range(NT // 8):
                    vps = pjps_pool.tile([128, 1024], F32, tag="pj")
                    for t8 in range(8):
                        nc.tensor.matmul(
                            vps[:, t8 * 128 : (t8 + 1) * 128],
                            vT[:, (b8 * 8 + t8) * 128 : (b8 * 8 + t8 + 1) * 128],
                            wvt[:, :],
                            start=(t8 == 0 or t8 == 4),  # first MM per PSUM bank
                            stop=False,
                        )
                    # += ones.T @ bv4  (adds bv[e] to every row)
                    nc.tensor.matmul(
                        vps[:, 0:512], ones1[:, :], bv4[:, :], start=False, stop=False
                    )
                    nc.tensor.matmul(
                        vps[:, 512:1024], ones1[:, :], bv4[:, :], start=False, stop=True
                    )
                    nc.vector.tensor_copy(
                        vpxv[:, b8 * 8 : (b8 + 1) * 8, 0:128],
                        vps[:, :].rearrange("p (t n) -> p t n", t=8),
                    )
                return vpx, vpxv

            def sme(qpT, kpT, ic):
                # S matmuls + mask-mul + exp for one i-chunk -> pt (P^T bf16)
                i0 = ic * IC
                pt = pt_pool.tile([128, NT * IC], BF16, tag="pt")
                for tp in range(NT // 2):  # jt pairs
                    smt = sm_pool.tile([128, 2 * IC], BF16, tag="smt")
                    for o in range(2):
                        jt = 2 * tp + o
                        st = sps_pool.tile([128, IC], F32, tag="s")
                        for h in range(IC // 512):
                            nc.tensor.matmul(
                                st[:, h * 512 : (h + 1) * 512],
                                kpT[:, jt * 128 : (jt + 1) * 128],
                                qpT[:, i0 + h * 512 : i0 + (h + 1) * 512],
                                start=True,
                                stop=True,
                            )
                        nc.vector.tensor_mul(
                            smt[:, o * IC : (o + 1) * IC],
                            st[:, :],
                            mmt[:, jt * T + i0 : jt * T + i0 + IC],
                        )
                    nc.scalar.activation(
                        pt[:, 2 * tp * IC : (2 * tp + 2) * IC],
                        smt[:, :],
                        AF.Exp,
                        bias=negc[:, :],
                        scale=float(SCALE),
                    )
                return pt

            def av(s, ic, pt, vpxv):
                ptv = pt[:, :].rearrange("p (j i) -> p j i", j=NT)
                ot8 = fin_pool.tile([128, IC // 128 * D], F32, tag="ot8")
                for itl in range(IC // 128):
                    io = itl * 128
                    ops = ops_pool.tile([128, 129], F32, tag="o")
                    for jt in range(NT):
                        nc.tensor.matmul(
                            ops[:, :],
                            ptv[:, jt, io : io + 128],
                            vpxv[:, jt, :],
                            start=(jt == 0),
                            stop=(jt == NT - 1),
                        )
                    rl = rl_pool.tile([128, 1], F32, tag="rl")
                    nc.vector.reciprocal(rl[:, :], ops[:, 128:129])
                    nc.scalar.mul(ot8[:, io : io + 128], ops[:, 0:128], rl[:, :])
                r0 = (s * NCI + ic) * 128
                nc.sync.dma_start(out=out_d[r0 : r0 + 128, :], in_=ot8[:, :])

            # pipeline: AV of slab s interleaves with loads/proj of slab s+1
            # and SME of the next chunk, keeping PE busy in Vector's shadow.
            qT, kT, vT = load(0)
            qpT, kpT = proj(qT, kT)
            vpx, vpxv = vproj(vT)
            pt0 = sme(qpT, kpT, 0)
            pt1 = sme(qpT, kpT, 1)
            for s in range(SPC):
                last = s == SPC - 1
                if last:
                    av(s, 0, pt0, vpxv)
                    av(s, 1, pt1, vpxv)
                    break
                qTn, kTn, vTn = load(s + 1)
                av(s, 0, pt0, vpxv)
                qpTn, kpTn = proj(qTn, kTn)
                vpxn, vpxvn = vproj(vTn)
                pt0n = sme(qpTn, kpTn, 0)
                av(s, 1, pt1, vpxv)
                pt1n = sme(qpTn, kpTn, 1)
                pt0, pt1 = pt0n, pt1n
                qpT, kpT, vpx, vpxv = qpTn, kpTn, vpxn, vpxvn
    if not nc.is_finalized():
        nc.finalize()
    return nc


_NC_CACHE = None


def kernel(q, k, v, mask, Wq, bq, Wk, bk, Wv, bv):
    global _NC_CACHE
    if _NC_CACHE is None:
        _NC_CACHE = _build_nc()
    nc = _NC_CACHE

    bf16 = ml_dtypes.bfloat16
    fp8 = ml_dtypes.float8_e4m3fn

    # host-side layout transforms (per-core slab-major, transposed, bf16)
    qf = np.asarray(q, np.float32).reshape(B * H, T, D)
    kf = np.asarray(k, np.float32).reshape(B * H, T, D)
    vf = np.asarray(v, np.float32).reshape(B * H, T, D)
    qt = np.ascontiguousarray(qf.transpose(0, 2, 1)).astype(bf16)  # [64,128,T]
    kt = np.ascontiguousarray(kf.transpose(0, 2, 1)).astype(bf16)
    vt = np.ascontiguousarray(vf.transpose(0, 2, 1)).astype(bf16)
    mmt = np.ascontiguousarray(
        (1.0 - np.asarray(mask, np.float32)[0, 0]).T
    ).astype(fp8)
    wqt = np.ascontiguousarray(np.asarray(Wq, np.float32).T).astype(bf16)
    wkt = np.ascontiguousarray(np.asarray(Wk, np.float32).T).astype(bf16)
    wvt = np.ascontiguousarray(np.asarray(Wv, np.float32).T).astype(bf16)
    bqc = np.asarray(bq, np.float32).reshape(D, 1).copy()
    bkc = np.asarray(bk, np.float32).reshape(D, 1).copy()
    bv4 = np.tile(np.asarray(bv, np.float32), 4).reshape(1, 512).astype(bf16)

    in_maps = []
    for c in range(NCORES):
        sl = slice(c * SPC, (c + 1) * SPC)
        in_maps.append(
            {
                "qt": np.ascontiguousarray(qt[sl].reshape(SPC * 128, T)),
                "kt": np.ascontiguousarray(kt[sl].reshape(SPC * 128, T)),
                "vt": np.ascontiguousarray(vt[sl].reshape(SPC * 128, T)),
                "mmt": mmt,
                "wqt": wqt,
                "wkt": wkt,
                "wvt": wvt,
                "bqc": bqc,
                "bkc": bkc,
                "bv4": bv4,
            }
        )

    global _LAST_IN_MAPS
    _LAST_IN_MAPS = in_maps
    res = run_bass_kernel_spmd(nc, in_maps, core_ids=list(range(NCORES)))
    # out blocks: [SPC*NCI*128, 8*128]: row=(s*NCI+ic)*128+p, col=t*128+e
    outs = [
        np.asarray(res.results[c]["out"]).reshape(SPC, NCI, 128, IC // 128, D)
        for c in range(NCORES)
    ]
    full = np.concatenate(outs, axis=0)  # [64, NCI, 128, 8, 128]
    # i = ic*1024 + t*128 + p  ->  order (s, ic, t, p, e)
    full = full.transpose(0, 1, 3, 2, 4).reshape(B, H, T, D)
    return np.ascontiguousarray(full).astype(np.float32)


# revision 11
# speedup vs baseline: 1.0790x; 1.0790x over previous
"""Distributed Bass attention kernel for trn2 (8 NeuronCores).

Problem: B=4,H=16,T=2048,D=128 attention w/ Q/K/V linear projections.
  qp = q@Wq.T+bq ; kp = k@Wk.T+bk ; vp = v@Wv.T+bv
  S = qp@kp.T/sqrt(128); S = where(mask==1, -1e-9, S); P=softmax(S); out = P@vp

Key identities:
  - masked logit -1e-9 ~= 0  =>  P_unnorm[i,j] = exp(S[i,j]*(1-m)) (exp(0)=1).
  - global shift C=ln(8): P' = exp(S*(1-m) - C) = P/8 cancels in softmax;
    masked entries become exactly 0.125 (representable in low precision).
  - out row i = (P' @ vpx)[i,:]/l[i], l via ones-column appended to vp.
  - bias bv folded into vp via a K=1 rank-1 matmul accumulation
    (ones[1,128].T @ bv4[1,512]); out = P_norm@(vp+bv) == P_norm@vp + bv.

Sharding: 64 (b,h) slabs -> 8 per core (head/data parallel, no collectives).

Per-core dataflow (host pre-transposes+casts q/k/v to [d,t] bf16):
  - qpT[e,t] = wqt.T @ qT (+bq per-partition during PSUM->SBUF activation)
  - vp[t,e] natural via lhsT=vT tile, rhs=wvt; +bv via rank-1 matmul;
    ones col appended -> vpx bf16 [128, 16*129]
  - S TRANSPOSED in [128,1024] 2-bank PSUM groups: ST[j,i] = kpT_j.T @ qpT_i
  - DVE: smt = ST * (1-m)^T (mask fp8 0/1, exact), bf16 out, [128,2048] pairs
  - Scalar: P^T = exp(smt*scale - C) -> bf16 pt, one [128,2048] instr per pair
  - AV: out[i,0:129] = sum_jt PT_tile.T @ vpx_tile (l in col 128)
  - epilogue: rl=1/l (DVE), out = O*rl on Scalar (PSUM->SBUF), chunked DMA out
"""

import numpy as np
import ml_dtypes

import sys
sys.path.insert(0, "/opt/trn_rl_repo")

from concourse import bacc, bass, mybir
from concourse.tile import TileContext
from concourse.bass_utils import run_bass_kernel_spmd

B, H, T, D = 4, 16, 2048, 128
NCORES = 8
SPC = (B * H) // NCORES  # 8 slabs per core
ROWS = SPC * T
NT = T // 128  # 16 j-tiles
IC = 1024  # i-chunk size
NCI = T // IC  # 2
SCALE = 1.0 / np.sqrt(D)
C_SHIFT = float(np.log(8.0))

F32 = mybir.dt.float32
BF16 = mybir.dt.bfloat16
FP8 = mybir.dt.float8e4
AF = mybir.ActivationFunctionType


def _build_nc():
    nc = bacc.Bacc(target_bir_lowering=False, trn_type="TRN2")

    qt_d = nc.declare_dram_parameter("qt", [SPC * 128, T], BF16, isOutput=False)
    kt_d = nc.declare_dram_parameter("kt", [SPC * 128, T], BF16, isOutput=False)
    vt_d = nc.declare_dram_parameter("vt", [SPC * 128, T], BF16, isOutput=False)
    mmt_d = nc.declare_dram_parameter("mmt", [T, T], FP8, isOutput=False)
    wqt_d = nc.declare_dram_parameter("wqt", [D, D], BF16, isOutput=False)
    wkt_d = nc.declare_dram_parameter("wkt", [D, D], BF16, isOutput=False)
    wvt_d = nc.declare_dram_parameter("wvt", [D, D], BF16, isOutput=False)
    bqc_d = nc.declare_dram_parameter("bqc", [D, 1], F32, isOutput=False)
    bkc_d = nc.declare_dram_parameter("bkc", [D, 1], F32, isOutput=False)
    bv4_d = nc.declare_dram_parameter("bv4", [1, 512], BF16, isOutput=False)
    # out blocks: row = (s*NCI + ic)*128 + p, col = t*128 + e  (host reshapes)
    out_d = nc.declare_dram_parameter(
        "out", [SPC * NCI * 128, IC // 128 * D], F32, isOutput=True
    )

    with TileContext(nc) as tc:
        with (
            tc.tile_pool(name="const", bufs=1) as const_pool,
            tc.tile_pool(name="mmt", bufs=1) as mmt_pool,
            tc.tile_pool(name="qkvt", bufs=2) as qkvt_pool,
            tc.tile_pool(name="proj", bufs=2) as proj_pool,
            tc.tile_pool(name="vpx", bufs=2) as vpx_pool,
            tc.tile_pool(name="sm", bufs=3) as sm_pool,
            tc.tile_pool(name="pt", bufs=2) as pt_pool,
            tc.tile_pool(name="fin", bufs=2) as fin_pool,
            tc.tile_pool(name="rl", bufs=8) as rl_pool,
            tc.tile_pool(name="pj_ps", bufs=1, space="PSUM") as pjps_pool,
            tc.tile_pool(name="s_ps", bufs=2, space="PSUM") as sps_pool,
            tc.tile_pool(name="o_ps", bufs=2, space="PSUM") as ops_pool,
        ):
            # ---- constants (once per core) ----
            wqt = const_pool.tile([128, 128], BF16, tag="wqt")
            nc.sync.dma_start(out=wqt[:, :], in_=wqt_d[:, :])
            wkt = const_pool.tile([128, 128], BF16, tag="wkt")
            nc.sync.dma_start(out=wkt[:, :], in_=wkt_d[:, :])
            wvt = const_pool.tile([128, 128], BF16, tag="wvt")
            nc.sync.dma_start(out=wvt[:, :], in_=wvt_d[:, :])
            bqc = const_pool.tile([128, 1], F32, tag="bqc")
            nc.sync.dma_start(out=bqc[:, :], in_=bqc_d[:, :])
            bkc = const_pool.tile([128, 1], F32, tag="bkc")
            nc.sync.dma_start(out=bkc[:, :], in_=bkc_d[:, :])
            bv4 = const_pool.tile([1, 512], BF16, tag="bv4")
            nc.sync.dma_start(out=bv4[:, :], in_=bv4_d[:, :])
            ones1 = const_pool.tile([1, 128], BF16, tag="ones1")
            nc.vector.memset(ones1[:, :], 1.0)
            negc = const_pool.tile([128, 1], F32, tag="negc")
            nc.vector.memset(negc[:, :], -C_SHIFT)

            # slab-0 q/k/v loads issued BEFORE the big mask load so the
            # proj->S->mul pipeline starts immediately
            qkv0 = []
            for name, src in (("qT", qt_d), ("kT", kt_d), ("vT", vt_d)):
                t0 = qkvt_pool.tile([128, T], BF16, tag=name)
                nc.sync.dma_start(out=t0[:, :], in_=src[0:128, :])
                qkv0.append(t0)

            # transposed mask multiplier (1-mask).T, fp8 (0/1 exact), [j, i]
            mmt = mmt_pool.tile([128, NT * T], FP8, tag="mmt")
            for jt in range(NT):
                nc.sync.dma_start(
                    out=mmt[:, jt * T : (jt + 1) * T],
                    in_=mmt_d[jt * 128 : (jt + 1) * 128, :],
                )

            # ---- software-pipelined slab phases ----
            def load(s):
                if s == 0:
                    return qkv0
                tiles = []
                for name, src in (("qT", qt_d), ("kT", kt_d), ("vT", vt_d)):
                    t = qkvt_pool.tile([128, T], BF16, tag=name)
                    nc.sync.dma_start(
                        out=t[:, :], in_=src[s * 128 : (s + 1) * 128, :]
                    )
                    tiles.append(t)
                return tiles

            def proj(qT, kT):
                qpT = proj_pool.tile([128, T], BF16, tag="qpT")
                kpT = proj_pool.tile([128, T], BF16, tag="kpT")
                for srcT, w, bias, dst in ((qT, wqt, bqc, qpT), (kT, wkt, bkc, kpT)):
                    for c in range(T // 1024):
                        pps = pjps_pool.tile([128, 1024], F32, tag="pj")
                        for h in range(2):
                            nc.tensor.matmul(
                                pps[:, h * 512 : (h + 1) * 512],
                                w[:, :],
                                srcT[:, c * 1024 + h * 512 : c * 1024 + (h + 1) * 512],
                                start=True,
                                stop=True,
                            )
                        nc.scalar.activation(
                            dst[:, c * 1024 : (c + 1) * 1024],
                            pps[:, :],
                            AF.Identity,
                            bias=bias[:, :],
                            scale=1.0,
                        )
                return qpT, kpT

            def vproj(vT):
                # vpx: 16 blocks [128(t), 129] bf16; col 128 = 1.0
                vpx = vpx_pool.tile([128, NT * 129], BF16, tag="vpx")
                nc.gpsimd.memset(vpx[:, :], 1.0)
                vpxv = vpx[:, :].rearrange("p (j n) -> p j n", j=NT)
                for b8 in range(NT // 8):
                    vps = pjps_pool.tile([128, 1024], F32, tag="pj")
                    for t8 in range(8):
                        nc.tensor.matmul(
                            vps[:, t8 * 128 : (t8 + 1) * 128],
                            vT[:, (b8 * 8 + t8) * 128 : (b8 * 8 + t8 + 1) * 128],
                            wvt[:, :],
                            start=(t8 == 0 or t8 == 4),  # first MM per PSUM bank
                            stop=False,
                        )
                    # += ones.T @ bv4  (adds bv[e] to every row)
                    nc.tensor.matmul(
                        vps[:, 0:512], ones1[:, :], bv4[:, :], start=False, stop=False
                    )
                    nc.tensor.matmul(
                        vps[:, 512:1024], ones1[:, :], bv4[:, :], start=False, stop=True
                    )
                    nc.vector.tensor_copy(
                        vpxv[:, b8 * 8 : (b8 + 1) * 8, 0:128],
                        vps[:, :].rearrange("p (t n) -> p t n", t=8),
                    )
                return vpx, vpxv

            def sme(qpT, kpT, ic):
                # S matmuls + mask-mul + exp for one i-chunk -> pt (P^T bf16)
                i0 = ic * IC
                pt = pt_pool.tile([128, NT * IC], BF16, tag="pt")
                for tp in range(NT // 2):  # jt pairs
                    smt = sm_pool.tile([128, 2 * IC], BF16, tag="smt")
                    for o in range(2):
                        jt = 2 * tp + o
                        st = sps_pool.tile([128, IC], F32, tag="s")
                        for h in range(IC // 512):
                            nc.tensor.matmul(
                                st[:, h * 512 : (h + 1) * 512],
                                kpT[:, jt * 128 : (jt + 1) * 128],
                                qpT[:, i0 + h * 512 : i0 + (h + 1) * 512],
                                start=True,
                                stop=True,
                            )
                        nc.vector.tensor_mul(
                            smt[:, o * IC : (o + 1) * IC],
                            st[:, :],
                            mmt[:, jt * T + i0 : jt * T + i0 + IC],
                        )
                    nc.scalar.activation(
                        pt[:, 2 * tp * IC : (2 * tp + 2) * IC],
                        smt[:, :],
                        AF.Exp,
                        bias=negc[:, :],
                        scale=float(SCALE),
                    )
                return pt

            def av(s, ic, pt, vpxv):
                ptv = pt[:, :].rearrange("p (j i) -> p j i", j=NT)
                ot8 = fin_pool.tile([128, IC // 128 * D], F32, tag="ot8")
                for itl in range(IC // 128):
                    io = itl * 128
                    ops = ops_pool.tile([128, 129], F32, tag="o")
                    for jt in range(NT):
                        nc.tensor.matmul(
                            ops[:, :],
                            ptv[:, jt, io : io + 128],
                            vpxv[:, jt, :],
                            start=(jt == 0),
                            stop=(jt == NT - 1),
                        )
                    rl = rl_pool.tile([128, 1], F32, tag="rl")
                    nc.vector.reciprocal(rl[:, :], ops[:, 128:129])
                    nc.scalar.mul(ot8[:, io : io + 128], ops[:, 0:128], rl[:, :])
                r0 = (s * NCI + ic) * 128
                nc.sync.dma_start(out=out_d[r0 : r0 + 128, :], in_=ot8[:, :])

            for s in range(SPC):
                qT, kT, vT = load(s)
                qpT, kpT = proj(qT, kT)
                vpx, vpxv = vproj(vT)
                for ic in range(NCI):
                    pt = sme(qpT, kpT, ic)
                    av(s, ic, pt, vpxv)
    if not nc.is_finalized():
        nc.finalize()
    return nc


_NC_CACHE = None


def kernel(q, k, v, mask, Wq, bq, Wk, bk, Wv, bv):
    global _NC_CACHE
    if _NC_CACHE is None:
        _NC_CACHE = _build_nc()
    nc = _NC_CACHE

    bf16 = ml_dtypes.bfloat16
    fp8 = ml_dtypes.float8_e4m3fn

    # host-side layout transforms (per-core slab-major, transposed, bf16)
    qf = np.asarray(q, np.float32).reshape(B * H, T, D)
    kf = np.asarray(k, np.float32).reshape(B * H, T, D)
    vf = np.asarray(v, np.float32).reshape(B * H, T, D)
    qt = np.ascontiguousarray(qf.transpose(0, 2, 1)).astype(bf16)  # [64,128,T]
    kt = np.ascontiguousarray(kf.transpose(0, 2, 1)).astype(bf16)
    vt = np.ascontiguousarray(vf.transpose(0, 2, 1)).astype(bf16)
    mmt = np.ascontiguousarray(
        (1.0 - np.asarray(mask, np.float32)[0, 0]).T
    ).astype(fp8)
    wqt = np.ascontiguousarray(np.asarray(Wq, np.float32).T).astype(bf16)
    wkt = np.ascontiguousarray(np.asarray(Wk, np.float32).T).astype(bf16)
    wvt = np.ascontiguousarray(np.asarray(Wv, np.float32).T).astype(bf16)
    bqc = np.asarray(bq, np.float32).reshape(D, 1).copy()
    bkc = np.asarray(bk, np.float32).reshape(D, 1).copy()
    bv4 = np.tile(np.asarray(bv, np.float32), 4).reshape(1, 512).astype(bf16)

    in_maps = []
    for c in range(NCORES):
        sl = slice(c * SPC, (c + 1) * SPC)
        in_maps.append(
            {
                "qt": np.ascontiguousarray(qt[sl].reshape(SPC * 128, T)),
                "kt": np.ascontiguousarray(kt[sl].reshape(SPC * 128, T)),
                "vt": np.ascontiguousarray(vt[sl].reshape(SPC * 128, T)),
                "mmt": mmt,
                "wqt": wqt,
                "wkt": wkt,
                "wvt": wvt,
                "bqc": bqc,
                "bkc": bkc,
                "bv4": bv4,
            }
        )

    global _LAST_IN_MAPS
    _LAST_IN_MAPS = in_maps
    res = run_bass_kernel_spmd(nc, in_maps, core_ids=list(range(NCORES)))
    # out blocks: [SPC*NCI*128, 8*128]: row=(s*NCI+ic)*128+p, col=t*128+e
    outs = [
        np.asarray(res.results[c]["out"]).reshape(SPC, NCI, 128, IC // 128, D)
        for c in range(NCORES)
    ]
    full = np.concatenate(outs, axis=0)  # [64, NCI, 128, 8, 128]
    # i = ic*1024 + t*128 + p  ->  order (s, ic, t, p, e)
    full = full.transpose(0, 1, 3, 2, 4).reshape(B, H, T, D)
    return np.ascontiguousarray(full).astype(np.float32)
